# revision 7
# baseline (speedup 1.0000x reference)
"""Multi-head attention (B=2, S=2048, E=1024, H=16, causal) on 8 Trainium2 cores.

Sharding: data-parallel over batch (2) x tensor-parallel over heads (4 groups
of 4 heads). Core i handles batch i//4, heads 4*(i%4) .. 4*(i%4)+3.
Each core computes Q/K/V projections for its 256 channels, causal
flash-attention for its 4 heads, and a partial output projection
(contribution of its channels to all 1024 output features). Partials are
summed across the 4 cores of each batch group (host-side by default).

All matmuls run as float32r (full-rate on TRN2 TensorE for N>=256).
"""
import numpy as np

import concourse.bass as bass
import concourse.tile as tile
from concourse import bacc, mybir
from concourse.bass_utils import run_bass_kernel_spmd

F32 = mybir.dt.float32
F32R = mybir.dt.float32r
ActF = mybir.ActivationFunctionType
Alu = mybir.AluOpType

B, S, E = 2, 2048, 1024
H, DH = 16, 64
NCORES, TPW = 8, 4          # 8 cores, 4-way tensor parallel per batch
HPC = H // TPW              # heads per core = 4
C = HPC * DH                # channels per core = 256
SCALE = 1.0 / 8.0           # 1/sqrt(DH)
VW = HPC * (DH + 1)         # V storage width per s-tile (ones col per head)
NST = S // 128              # 16 s-tiles of 128 rows
NQB = S // 512              # 4 q-blocks of 512
NEC = E // 128              # 8 e-chunks (contraction for projections)

_cache = {}


def _emit(nc, tc, causal):
    # ---- DRAM parameters ----
    xt_d = nc.dram_tensor("xt", [E, S], F32, kind="ExternalInput").ap()
    wqt_d = nc.dram_tensor("wqt", [E, C], F32, kind="ExternalInput").ap()
    wkt_d = nc.dram_tensor("wkt", [E, C], F32, kind="ExternalInput").ap()
    wvt_d = nc.dram_tensor("wvt", [E, C], F32, kind="ExternalInput").ap()
    wot_d = nc.dram_tensor("wot", [C, E], F32, kind="ExternalInput").ap()
    bqk_d = nc.dram_tensor("bqk", [128, 4], F32, kind="ExternalInput").ap()
    bv_d = nc.dram_tensor("bv", [1, C], F32, kind="ExternalInput").ap()
    bo_d = nc.dram_tensor("bo", [1, E], F32, kind="ExternalInput").ap()
    ones_d = nc.dram_tensor("ones", [1, 128], F32, kind="ExternalInput").ap()
    onesv_d = nc.dram_tensor("onesv", [128, NST * HPC], F32, kind="ExternalInput").ap()
    out_d = nc.dram_tensor("out", [S, E], F32, kind="ExternalOutput").ap()

    ctxpool = tc.tile_pool

    with ctxpool(name="persist", bufs=1) as pp:
        # ---- persistent SBUF tensors ----
        xt_sb = pp.tile([128, NEC * S], F32R)       # X^T, e-chunk ec at cols [ec*S)
        wqt_sb = pp.tile([128, NEC * C], F32R)
        wkt_sb = pp.tile([128, NEC * C], F32R)
        wvt_sb = pp.tile([128, NEC * C], F32R)
        wot_sb = pp.tile([128, 2 * E], F32R)        # c-chunk cc at cols [cc*E)
        qt_sb = pp.tile([128, 2 * S], F32R)         # Q^T, d-tile t at cols [t*S)
        kt_sb = pp.tile([128, 2 * S], F32R)
        v_sb = pp.tile([128, NST * VW], F32R)       # V (+ones col per head)
        ot_sb = pp.tile([128, 2 * S], F32R)         # normalized attn out^T
        bqk_sb = pp.tile([128, 4], F32)
        bvb_sb = pp.tile([128, C], F32)             # bv broadcast to partitions
        bob_sb = pp.tile([128, E], F32)             # bo broadcast to partitions
        ones_sb = pp.tile([1, 128], F32R)

        # ---- input DMAs ----
        for ec in range(NEC):
            nc.sync.dma_start(out=wqt_sb[:, ec * C:(ec + 1) * C],
                              in_=wqt_d[ec * 128:(ec + 1) * 128, :].bitcast(F32R))
            nc.sync.dma_start(out=wkt_sb[:, ec * C:(ec + 1) * C],
                              in_=wkt_d[ec * 128:(ec + 1) * 128, :].bitcast(F32R))
        for ec in range(NEC):
            nc.sync.dma_start(out=xt_sb[:, ec * S:(ec + 1) * S],
                              in_=xt_d[ec * 128:(ec + 1) * 128, :].bitcast(F32R))
        for ec in range(NEC):
            nc.sync.dma_start(out=wvt_sb[:, ec * C:(ec + 1) * C],
                              in_=wvt_d[ec * 128:(ec + 1) * 128, :].bitcast(F32R))
        for cc in range(2):
            nc.sync.dma_start(out=wot_sb[:, cc * E:(cc + 1) * E],
                              in_=wot_d[cc * 128:(cc + 1) * 128, :].bitcast(F32R))
        nc.sync.dma_start(out=bqk_sb[:], in_=bqk_d[:])
        nc.sync.dma_start(out=ones_sb[:], in_=ones_d[:].bitcast(F32R))
        # ones columns of V: one strided DMA over all s-tiles/heads
        v_ones_ap = v_sb[:].rearrange("p (n x) -> p n x", x=DH + 1)[:, :, DH:DH + 1]
        nc.sync.dma_start(
            out=v_ones_ap,
            in_=onesv_d[:].bitcast(F32R).rearrange("p (n x) -> p n x", x=1))

        with ctxpool(name="small", bufs=1) as sp:
            bv_row = sp.tile([1, C], F32R)
            bo_row = sp.tile([1, E], F32R)
            nc.sync.dma_start(out=bv_row[:], in_=bv_d[:].bitcast(F32R))
            nc.sync.dma_start(out=bo_row[:], in_=bo_d[:].bitcast(F32R))

            # ==== phase B: projections ====
            with ctxpool(name="proj_ps", bufs=2, space="PSUM") as proj_ps, \
                 ctxpool(name="vproj_ps", bufs=2, space="PSUM") as vproj_ps:
                # bias broadcasts via K=1 matmul against ones
                ps_bv = vproj_ps.tile([128, C], F32)
                nc.tensor.matmul(ps_bv[:], ones_sb[0:1, 0:128], bv_row[:],
                                 start=True, stop=True)
                nc.vector.tensor_copy(bvb_sb[:], ps_bv[:])
                for eb in range(2):
                    ps_bo = proj_ps.tile([128, 512], F32)
                    nc.tensor.matmul(ps_bo[:], ones_sb[0:1, 0:128],
                                     bo_row[0:1, eb * 512:(eb + 1) * 512],
                                     start=True, stop=True)
                    nc.vector.tensor_copy(bob_sb[:, eb * 512:(eb + 1) * 512], ps_bo[:])

                # Q^T / K^T projections: out[d(128), s(512)] = sum_e W^T-chunk.T @ X^T-chunk
                for dt in range(2):
                    for pj, (w_sb, o_sb, bcol) in enumerate(
                            [(wqt_sb, qt_sb, 0), (wkt_sb, kt_sb, 2)]):
                        for sb_i in range(NQB):
                            ps = proj_ps.tile([128, 512], F32)
                            for ec in range(NEC):
                                nc.tensor.matmul(
                                    ps[:],
                                    w_sb[:, ec * C + dt * 128: ec * C + dt * 128 + 128],
                                    xt_sb[:, ec * S + sb_i * 512: ec * S + sb_i * 512 + 512],
                                    start=(ec == 0), stop=(ec == NEC - 1))
                            nc.vector.tensor_scalar_add(
                                o_sb[:, dt * S + sb_i * 512: dt * S + sb_i * 512 + 512],
                                ps[:], bqk_sb[:, bcol + dt: bcol + dt + 1])

                # V projection: out[s(128), c(256)] = sum_e X^T-chunk.T @ Wv^T-chunk
                for st in range(NST):
                    ps = vproj_ps.tile([128, C], F32)
                    for ec in range(NEC):
                        nc.tensor.matmul(
                            ps[:],
                            xt_sb[:, ec * S + st * 128: ec * S + st * 128 + 128],
                            wvt_sb[:, ec * C: (ec + 1) * C],
                            start=(ec == 0), stop=(ec == NEC - 1))
                    dst = v_sb[:, st * VW: st * VW + VW].rearrange(
                        "p (h x) -> p h x", h=HPC)[:, :, 0:DH]
                    nc.vector.tensor_add(
                        dst,
                        ps[:].rearrange("p (h x) -> p h x", h=HPC),
                        bvb_sb[:].rearrange("p (h x) -> p h x", h=HPC))

            # ==== phase C: attention (q-block outer, head inner) + out-proj ====
            with ctxpool(name="score_ps", bufs=2, space="PSUM") as score_ps, \
                 ctxpool(name="attn_ps", bufs=2, space="PSUM") as attn_ps, \
                 ctxpool(name="misc_ps", bufs=2, space="PSUM") as misc_ps, \
                 ctxpool(name="pt_pool", bufs=2) as pt_pool, \
                 ctxpool(name="rec_pool", bufs=2) as rec_pool, \
                 ctxpool(name="bc_pool", bufs=2) as bc_pool, \
                 ctxpool(name="out_pool", bufs=2) as out_pool:
                for qb in range(NQB):
                    nk = 4 * (qb + 1) if causal else NST
                    q0 = qb * 512
                    for h in range(HPC):
                        t = h // 2
                        p0 = (h % 2) * 64
                        ps_o = attn_ps.tile([65, 512], F32)
                        for pr in range(nk // 2):
                            ps_s = score_ps.tile([128, 1024], F32)
                            pt = pt_pool.tile([128, 1024], F32R)
                            for u in range(2):
                                kt_i = 2 * pr + u
                                nc.tensor.matmul(
                                    ps_s[:, u * 512:(u + 1) * 512],
                                    kt_sb[p0:p0 + 64,
                                          t * S + kt_i * 128: t * S + kt_i * 128 + 128],
                                    qt_sb[p0:p0 + 64, t * S + q0: t * S + q0 + 512],
                                    start=True, stop=True)
                            nc.scalar.activation(pt[:], ps_s[:], ActF.Exp, scale=SCALE)
                            if causal:
                                for u in range(2):
                                    kt_i = 2 * pr + u
                                    off = kt_i * 128 - q0
                                    if off + 127 >= 0:  # tile crosses the diagonal
                                        nc.gpsimd.affine_select(
                                            out=pt[:, u * 512:(u + 1) * 512],
                                            in_=pt[:, u * 512:(u + 1) * 512],
                                            compare_op=Alu.is_ge,
                                            fill=0.0, base=-off,
                                            pattern=[[1, 512]],
                                            channel_multiplier=-1)
                            for u in range(2):
                                kt_i = 2 * pr + u
                                nc.tensor.matmul(
                                    ps_o[:],
                                    v_sb[:, kt_i * VW + h * (DH + 1):
                                         kt_i * VW + h * (DH + 1) + DH + 1],
                                    pt[:, u * 512:(u + 1) * 512],
                                    start=(kt_i == 0), stop=(kt_i == nk - 1),
                                    skip_group_check=True)
                        rec = rec_pool.tile([1, 512], F32R)
                        with nc.allow_low_precision(reason="f32r recip for PE bcast"):
                            nc.vector.reciprocal(rec[:], ps_o[64:65, :])
                        ps_b = misc_ps.tile([64, 512], F32, tag="mps")
                        nc.tensor.matmul(ps_b[:], ones_sb[0:1, 0:64], rec[:],
                                         start=True, stop=True)
                        bc = bc_pool.tile([64, 512], F32)
                        nc.vector.tensor_copy(bc[:], ps_b[:])
                        nc.vector.tensor_mul(
                            ot_sb[p0:p0 + 64, t * S + q0: t * S + q0 + 512],
                            ps_o[0:64, :], bc[:])

                    # out-projection for this q-block's 4 s-tiles
                    for st in range(qb * 4, qb * 4 + 4):
                        o_t = out_pool.tile([128, E], F32)
                        for eb in range(2):
                            ps_f = misc_ps.tile([128, 512], F32, tag="mps")
                            for cc in range(2):
                                nc.tensor.matmul(
                                    ps_f[:],
                                    ot_sb[:, cc * S + st * 128: cc * S + st * 128 + 128],
                                    wot_sb[:, cc * E + eb * 512: cc * E + eb * 512 + 512],
                                    start=(cc == 0), stop=(cc == 1))
                            nc.vector.tensor_add(
                                o_t[:, eb * 512:(eb + 1) * 512], ps_f[:],
                                bob_sb[:, eb * 512:(eb + 1) * 512])
                        nc.sync.dma_start(
                            out=out_d[st * 128:(st + 1) * 128, :], in_=o_t[:])


def _build(causal):
    nc = bacc.Bacc("TRN2", target_bir_lowering=False, debug=False,
                   num_devices=NCORES)
    with tile.TileContext(nc) as tc:
        _emit(nc, tc, causal)
    nc.compile()
    return nc


def _shard_inputs(QKV, Wq, bq, Wk, bk, Wv, bv, Wo, bo):
    QKV = np.asarray(QKV, dtype=np.float32)
    Wq, Wk, Wv, Wo = (np.asarray(w, dtype=np.float32) for w in (Wq, Wk, Wv, Wo))
    bq, bk, bv, bo = (np.asarray(b_, dtype=np.float32) for b_ in (bq, bk, bv, bo))
    ones = np.ones((1, 128), dtype=np.float32)
    onesv = np.ones((128, NST * HPC), dtype=np.float32)
    in_maps = []
    for core in range(NCORES):
        b, g = divmod(core, TPW)
        cs = slice(g * C, (g + 1) * C)
        bqs, bks = bq[cs], bk[cs]
        bqk = np.stack([bqs[:128], bqs[128:], bks[:128], bks[128:]], axis=1)
        in_maps.append({
            "xt": np.ascontiguousarray(QKV[b].T),
            "wqt": np.ascontiguousarray(Wq[cs, :].T),
            "wkt": np.ascontiguousarray(Wk[cs, :].T),
            "wvt": np.ascontiguousarray(Wv[cs, :].T),
            "wot": np.ascontiguousarray(Wo[:, cs].T),
            "bqk": np.ascontiguousarray(bqk),
            "bv": bv[cs].reshape(1, C).copy(),
            # host sums the 4 tensor-parallel partials per batch; only one
            # core per group contributes the output bias
            "bo": (bo if g == 0 else np.zeros_like(bo)).reshape(1, E).copy(),
            "ones": ones,
            "onesv": onesv,
        })
    return in_maps


def kernel(QKV, Wq, bq, Wk, bk, Wv, bv, Wo, bo, is_causal):
    causal = bool(int(np.asarray(is_causal)))
    if causal not in _cache:
        _cache[causal] = _build(causal)
    nc = _cache[causal]
    in_maps = _shard_inputs(QKV, Wq, bq, Wk, bk, Wv, bv, Wo, bo)
    res = run_bass_kernel_spmd(nc, in_maps, core_ids=list(range(NCORES)))
    out = np.empty((B, S, E), dtype=np.float32)
    for b in range(B):
        acc = res.results[TPW * b]["out"].astype(np.float32)
        for g in range(1, TPW):
            acc = acc + res.results[TPW * b + g]["out"]
        out[b] = acc
    return out


# revision 9
# speedup vs baseline: 1.0822x; 1.0822x over previous
"""Multi-head attention (B=2, S=2048, E=1024, H=16, causal) on 8 Trainium2 cores.

Sharding: data-parallel over batch (2) x tensor-parallel over heads (4 groups
of 4 heads). Core i handles batch i//4, heads 4*(i%4) .. 4*(i%4)+3.
Each core computes Q/K/V projections for its 256 channels, causal
flash-attention for its 4 heads, and a partial output projection
(contribution of its channels to all 1024 output features). Partials are
summed across the 4 cores of each batch group (host-side).

All big matmuls run as float32r (full-rate on TRN2 TensorE for N>=256).
V projection is deferred per-q-block to give the PE filler work while the
ACT engine paces the softmax inner loop (keeps HAM from throttling).
"""
import numpy as np

import concourse.bass as bass
import concourse.tile as tile
from concourse import bacc, mybir
from concourse.bass_utils import run_bass_kernel_spmd

F32 = mybir.dt.float32
F32R = mybir.dt.float32r
ActF = mybir.ActivationFunctionType
Alu = mybir.AluOpType

B, S, E = 2, 2048, 1024
H, DH = 16, 64
NCORES, TPW = 8, 4          # 8 cores, 4-way tensor parallel per batch
HPC = H // TPW              # heads per core = 4
C = HPC * DH                # channels per core = 256
SCALE = 1.0 / 8.0           # 1/sqrt(DH)
VW = HPC * (DH + 1)         # V storage width per s-tile (ones col per head)
NST = S // 128              # 16 s-tiles of 128 rows
NQB = S // 512              # 4 q-blocks of 512
NEC = E // 128              # 8 e-chunks (contraction for projections)

_cache = {}


def _emit(nc, tc, causal):
    # ---- DRAM parameters ----
    xt_d = nc.dram_tensor("xt", [E, S], F32, kind="ExternalInput").ap()
    wqt_d = nc.dram_tensor("wqt", [E, C], F32, kind="ExternalInput").ap()
    wkt_d = nc.dram_tensor("wkt", [E, C], F32, kind="ExternalInput").ap()
    wvt_d = nc.dram_tensor("wvt", [E, C], F32, kind="ExternalInput").ap()
    wot_d = nc.dram_tensor("wot", [C, E], F32, kind="ExternalInput").ap()
    bqk_d = nc.dram_tensor("bqk", [128, 4], F32, kind="ExternalInput").ap()
    bv_d = nc.dram_tensor("bv", [1, C], F32, kind="ExternalInput").ap()
    bo_d = nc.dram_tensor("bo", [1, E], F32, kind="ExternalInput").ap()
    ones_d = nc.dram_tensor("ones", [1, 128], F32, kind="ExternalInput").ap()
    onesv_d = nc.dram_tensor("onesv", [128, NST * HPC], F32, kind="ExternalInput").ap()
    out_d = nc.dram_tensor("out", [S, E], F32, kind="ExternalOutput").ap()

    ctxpool = tc.tile_pool

    def emit_vproj(vproj_pool, st):
        """Project V for s-tile st into v_sb (with the per-head ones column)."""
        ps = vproj_pool.tile([128, C], F32, tag="mps")
        for ec in range(NEC):
            nc.tensor.matmul(
                ps[:],
                xt_sb[:, ec * S + st * 128: ec * S + st * 128 + 128],
                wvt_sb[:, ec * C: (ec + 1) * C],
                start=(ec == 0), stop=(ec == NEC - 1),
                skip_group_check=True)
        dst = v_sb[:, st * VW: st * VW + VW].rearrange(
            "p (h x) -> p h x", h=HPC)[:, :, 0:DH]
        nc.vector.tensor_add(
            dst,
            ps[:].rearrange("p (h x) -> p h x", h=HPC),
            bvb_sb[:].rearrange("p (h x) -> p h x", h=HPC))

    with ctxpool(name="persist", bufs=1) as pp:
        # ---- persistent SBUF tensors ----
        xt_sb = pp.tile([128, NEC * S], F32R)       # X^T, e-chunk ec at cols [ec*S)
        wvt_sb = pp.tile([128, NEC * C], F32R)
        wot_sb = pp.tile([128, 2 * E], F32R)        # c-chunk cc at cols [cc*E)
        qt_sb = pp.tile([128, 2 * S], F32R)         # Q^T, d-tile t at cols [t*S)
        kt_sb = pp.tile([128, 2 * S], F32R)
        v_sb = pp.tile([128, NST * VW], F32R)       # V (+ones col per head)
        ot_sb = pp.tile([128, 2 * S], F32R)         # normalized attn out^T
        bqk_sb = pp.tile([128, 4], F32)
        bvb_sb = pp.tile([128, C], F32)             # bv broadcast to partitions
        bob_sb = pp.tile([128, E], F32)             # bo broadcast to partitions
        ones_sb = pp.tile([1, 128], F32)

        with ctxpool(name="qkw", bufs=1) as qkw:
            wqt_sb = qkw.tile([128, NEC * C], F32R)
            wkt_sb = qkw.tile([128, NEC * C], F32R)

            # ---- input DMAs ----
            for ec in range(NEC):
                nc.sync.dma_start(out=wqt_sb[:, ec * C:(ec + 1) * C],
                                  in_=wqt_d[ec * 128:(ec + 1) * 128, :].bitcast(F32R))
                nc.sync.dma_start(out=wkt_sb[:, ec * C:(ec + 1) * C],
                                  in_=wkt_d[ec * 128:(ec + 1) * 128, :].bitcast(F32R))
            for ec in range(NEC):
                nc.sync.dma_start(out=xt_sb[:, ec * S:(ec + 1) * S],
                                  in_=xt_d[ec * 128:(ec + 1) * 128, :].bitcast(F32R))
            for ec in range(NEC):
                nc.sync.dma_start(out=wvt_sb[:, ec * C:(ec + 1) * C],
                                  in_=wvt_d[ec * 128:(ec + 1) * 128, :].bitcast(F32R))
            for cc in range(2):
                nc.sync.dma_start(out=wot_sb[:, cc * E:(cc + 1) * E],
                                  in_=wot_d[cc * 128:(cc + 1) * 128, :].bitcast(F32R))
            nc.sync.dma_start(out=bqk_sb[:], in_=bqk_d[:])
            nc.sync.dma_start(out=ones_sb[:], in_=ones_d[:])
            # ones columns of V: one strided DMA over all s-tiles/heads
            v_ones_ap = v_sb[:].rearrange("p (n x) -> p n x", x=DH + 1)[:, :, DH:DH + 1]
            nc.sync.dma_start(
                out=v_ones_ap,
                in_=onesv_d[:].bitcast(F32R).rearrange("p (n x) -> p n x", x=1))

            with ctxpool(name="small", bufs=1) as sp:
                bv_row = sp.tile([1, C], F32)
                bo_row = sp.tile([1, E], F32)
                nc.sync.dma_start(out=bv_row[:], in_=bv_d[:])
                nc.sync.dma_start(out=bo_row[:], in_=bo_d[:])

                # ==== phase B: bias broadcasts + Q^T/K^T projections ====
                with ctxpool(name="proj_ps", bufs=4, space="PSUM") as proj_ps:
                    # bias broadcasts via K=1 fp32 matmul against ones
                    ps_bv = proj_ps.tile([128, C], F32, tag="pps")
                    nc.tensor.matmul(ps_bv[:], ones_sb[0:1, 0:128], bv_row[:],
                                     start=True, stop=True)
                    nc.vector.tensor_copy(bvb_sb[:], ps_bv[:])
                    for eb in range(2):
                        ps_bo = proj_ps.tile([128, 512], F32, tag="pps")
                        nc.tensor.matmul(ps_bo[:], ones_sb[0:1, 0:128],
                                         bo_row[0:1, eb * 512:(eb + 1) * 512],
                                         start=True, stop=True)
                        nc.vector.tensor_copy(bob_sb[:, eb * 512:(eb + 1) * 512],
                                              ps_bo[:])

                    # Q^T/K^T: e-chunk-outer accumulation so the PE starts as
                    # soon as the first X^T chunk lands
                    for dt in range(2):
                        for w_sb, o_sb, bcol in ((wqt_sb, qt_sb, 0),
                                                 (wkt_sb, kt_sb, 2)):
                            pss = [proj_ps.tile([128, 512], F32, tag="pps",
                                                name=f"pp_{dt}_{bcol}_{i}")
                                   for i in range(NQB)]
                            for ec in range(NEC):
                                for sb_i in range(NQB):
                                    nc.tensor.matmul(
                                        pss[sb_i][:],
                                        w_sb[:, ec * C + dt * 128:
                                             ec * C + dt * 128 + 128],
                                        xt_sb[:, ec * S + sb_i * 512:
                                              ec * S + sb_i * 512 + 512],
                                        start=(ec == 0), stop=(ec == NEC - 1),
                                        skip_group_check=True)
                            for sb_i in range(NQB):
                                nc.vector.tensor_scalar_add(
                                    o_sb[:, dt * S + sb_i * 512:
                                         dt * S + sb_i * 512 + 512],
                                    pss[sb_i][:],
                                    bqk_sb[:, bcol + dt: bcol + dt + 1])

        # ==== phase C: attention (q-block outer, head inner) + out-proj ====
        with ctxpool(name="score_ps", bufs=2, space="PSUM") as score_ps, \
             ctxpool(name="attn_ps", bufs=2, space="PSUM") as attn_ps, \
             ctxpool(name="misc_ps", bufs=2, space="PSUM") as misc_ps, \
             ctxpool(name="pt_pool", bufs=3) as pt_pool, \
             ctxpool(name="rec_pool", bufs=2) as rec_pool, \
             ctxpool(name="bc_pool", bufs=2) as bc_pool, \
             ctxpool(name="out_pool", bufs=2) as out_pool:
            # V for the first q-block
            for st in range(4):
                emit_vproj(misc_ps, st)

            for qb in range(NQB):
                nk = 4 * (qb + 1) if causal else NST
                q0 = qb * 512
                for h in range(HPC):
                    t = h // 2
                    p0 = (h % 2) * 64
                    ps_o = attn_ps.tile([65, 512], F32)
                    for pr in range(nk // 2):
                        ps_s = score_ps.tile([128, 1024], F32)
                        pt = pt_pool.tile([128, 1024], F32R)
                        for u in range(2):
                            kt_i = 2 * pr + u
                            nc.tensor.matmul(
                                ps_s[:, u * 512:(u + 1) * 512],
                                kt_sb[p0:p0 + 64,
                                      t * S + kt_i * 128: t * S + kt_i * 128 + 128],
                                qt_sb[p0:p0 + 64, t * S + q0: t * S + q0 + 512],
                                start=True, stop=True)
                        nc.scalar.activation(pt[:], ps_s[:], ActF.Exp, scale=SCALE)
                        if causal:
                            for u in range(2):
                                kt_i = 2 * pr + u
                                off = kt_i * 128 - q0
                                if off + 127 >= 0:  # tile crosses the diagonal
                                    nc.gpsimd.affine_select(
                                        out=pt[:, u * 512:(u + 1) * 512],
                                        in_=pt[:, u * 512:(u + 1) * 512],
                                        compare_op=Alu.is_ge,
                                        fill=0.0, base=-off,
                                        pattern=[[1, 512]],
                                        channel_multiplier=-1)
                        for u in range(2):
                            kt_i = 2 * pr + u
                            nc.tensor.matmul(
                                ps_o[:],
                                v_sb[:, kt_i * VW + h * (DH + 1):
                                     kt_i * VW + h * (DH + 1) + DH + 1],
                                pt[:, u * 512:(u + 1) * 512],
                                start=(kt_i == 0), stop=(kt_i == nk - 1),
                                skip_group_check=True)
                    # normalize: rowsum -> broadcast (PE) -> 1/x (DVE approx) -> mul
                    rs = rec_pool.tile([1, 512], F32)
                    nc.vector.tensor_copy(rs[:], ps_o[64:65, :])
                    ps_b = misc_ps.tile([64, 512], F32, tag="mps")
                    nc.tensor.matmul(ps_b[:], ones_sb[0:1, 0:64], rs[:],
                                     start=True, stop=True)
                    bc = bc_pool.tile([64, 512], F32)
                    nc.vector.reciprocal_approx_fast(bc[:], ps_b[:])
                    nc.vector.tensor_mul(
                        ot_sb[p0:p0 + 64, t * S + q0: t * S + q0 + 512],
                        ps_o[0:64, :], bc[:])

                # V projection for the next q-block (PE filler during softmax)
                if qb + 1 < NQB:
                    for st in range(4 * (qb + 1), 4 * (qb + 2)):
                        emit_vproj(misc_ps, st)

                # out-projection for this q-block's 4 s-tiles
                for st in range(qb * 4, qb * 4 + 4):
                    o_t = out_pool.tile([128, E], F32)
                    for eb in range(2):
                        ps_f = misc_ps.tile([128, 512], F32, tag="mps")
                        for cc in range(2):
                            nc.tensor.matmul(
                                ps_f[:],
                                ot_sb[:, cc * S + st * 128: cc * S + st * 128 + 128],
                                wot_sb[:, cc * E + eb * 512: cc * E + eb * 512 + 512],
                                start=(cc == 0), stop=(cc == 1))
                        nc.vector.tensor_add(
                            o_t[:, eb * 512:(eb + 1) * 512], ps_f[:],
                            bob_sb[:, eb * 512:(eb + 1) * 512])
                    nc.sync.dma_start(
                        out=out_d[st * 128:(st + 1) * 128, :], in_=o_t[:])


def _build(causal):
    nc = bacc.Bacc("TRN2", target_bir_lowering=False, debug=False,
                   num_devices=NCORES)
    with tile.TileContext(nc) as tc:
        _emit(nc, tc, causal)
    nc.compile()
    return nc


def _shard_inputs(QKV, Wq, bq, Wk, bk, Wv, bv, Wo, bo):
    QKV = np.asarray(QKV, dtype=np.float32)
    Wq, Wk, Wv, Wo = (np.asarray(w, dtype=np.float32) for w in (Wq, Wk, Wv, Wo))
    bq, bk, bv, bo = (np.asarray(b_, dtype=np.float32) for b_ in (bq, bk, bv, bo))
    ones = np.ones((1, 128), dtype=np.float32)
    onesv = np.ones((128, NST * HPC), dtype=np.float32)
    in_maps = []
    for core in range(NCORES):
        b, g = divmod(core, TPW)
        cs = slice(g * C, (g + 1) * C)
        bqs, bks = bq[cs], bk[cs]
        bqk = np.stack([bqs[:128], bqs[128:], bks[:128], bks[128:]], axis=1)
        in_maps.append({
            "xt": np.ascontiguousarray(QKV[b].T),
            "wqt": np.ascontiguousarray(Wq[cs, :].T),
            "wkt": np.ascontiguousarray(Wk[cs, :].T),
            "wvt": np.ascontiguousarray(Wv[cs, :].T),
            "wot": np.ascontiguousarray(Wo[:, cs].T),
            "bqk": np.ascontiguousarray(bqk),
            "bv": bv[cs].reshape(1, C).copy(),
            # host sums the 4 tensor-parallel partials per batch; only one
            # core per group contributes the output bias
            "bo": (bo if g == 0 else np.zeros_like(bo)).reshape(1, E).copy(),
            "ones": ones,
            "onesv": onesv,
        })
    return in_maps


def kernel(QKV, Wq, bq, Wk, bk, Wv, bv, Wo, bo, is_causal):
    causal = bool(int(np.asarray(is_causal)))
    if causal not in _cache:
        _cache[causal] = _build(causal)
    nc = _cache[causal]
    in_maps = _shard_inputs(QKV, Wq, bq, Wk, bk, Wv, bv, Wo, bo)
    res = run_bass_kernel_spmd(nc, in_maps, core_ids=list(range(NCORES)))
    out = np.empty((B, S, E), dtype=np.float32)
    for b in range(B):
        acc = res.results[TPW * b]["out"].astype(np.float32)
        for g in range(1, TPW):
            acc = acc + res.results[TPW * b + g]["out"]
        out[b] = acc
    return out


# revision 11
# speedup vs baseline: 1.1499x; 1.0626x over previous
"""Multi-head attention (B=2, S=2048, E=1024, H=16, causal) on 8 Trainium2 cores.

Sharding: data-parallel over batch (2) x tensor-parallel over heads (4 groups
of 4 heads). Core i handles batch i//4, heads 4*(i%4) .. 4*(i%4)+3.
Each core computes Q/K/V projections for its 256 channels, causal
flash-attention for its 4 heads, and a partial output projection
(contribution of its channels to all 1024 output features). Partials are
summed across the 4 cores of each batch group (host-side).

All big matmuls run as float32r (full-rate on TRN2 TensorE for N>=256).
V projection is deferred per-q-block to give the PE filler work while the
ACT engine paces the softmax inner loop (keeps HAM from throttling).
"""
import numpy as np

import concourse.bass as bass
import concourse.tile as tile
from concourse import bacc, mybir
from concourse.bass_utils import run_bass_kernel_spmd

F32 = mybir.dt.float32
F32R = mybir.dt.float32r
ActF = mybir.ActivationFunctionType
Alu = mybir.AluOpType

B, S, E = 2, 2048, 1024
H, DH = 16, 64
NCORES, TPW = 8, 4          # 8 cores, 4-way tensor parallel per batch
HPC = H // TPW              # heads per core = 4
C = HPC * DH                # channels per core = 256
SCALE = 1.0 / 8.0           # 1/sqrt(DH)
VW = HPC * (DH + 1)         # V storage width per s-tile (ones col per head)
NST = S // 128              # 16 s-tiles of 128 rows
NQB = S // 512              # 4 q-blocks of 512
NEC = E // 128              # 8 e-chunks (contraction for projections)

_cache = {}


def _emit(nc, tc, causal):
    # ---- DRAM parameters ----
    xt_d = nc.dram_tensor("xt", [E, S], F32, kind="ExternalInput").ap()
    wqt_d = nc.dram_tensor("wqt", [E, C], F32, kind="ExternalInput").ap()
    wkt_d = nc.dram_tensor("wkt", [E, C], F32, kind="ExternalInput").ap()
    wvt_d = nc.dram_tensor("wvt", [E, C], F32, kind="ExternalInput").ap()
    wot_d = nc.dram_tensor("wot", [C, E], F32, kind="ExternalInput").ap()
    bqk_d = nc.dram_tensor("bqk", [128, 4], F32, kind="ExternalInput").ap()
    bv_d = nc.dram_tensor("bv", [1, C], F32, kind="ExternalInput").ap()
    bo_d = nc.dram_tensor("bo", [1, E], F32, kind="ExternalInput").ap()
    ones_d = nc.dram_tensor("ones", [1, 128], F32, kind="ExternalInput").ap()
    onesv_d = nc.dram_tensor("onesv", [128, NST * HPC], F32, kind="ExternalInput").ap()
    out_d = nc.dram_tensor("out", [S, E], F32, kind="ExternalOutput").ap()

    ctxpool = tc.tile_pool

    with ctxpool(name="persist", bufs=1) as pp:
        # ---- persistent SBUF tensors ----
        xt_sb = pp.tile([128, NEC * S], F32R)       # X^T, e-chunk ec at cols [ec*S)
        wvt_sb = pp.tile([128, NEC * C], F32R)
        wot_sb = pp.tile([128, 2 * E], F32R)        # c-chunk cc at cols [cc*E)
        qt_sb = pp.tile([128, 2 * S], F32R)         # Q^T, d-tile t at cols [t*S)
        kt_sb = pp.tile([128, 2 * S], F32R)
        v_sb = pp.tile([128, NST * VW], F32R)       # V (+ones col per head)
        ot_sb = pp.tile([128, 2 * S], F32R)         # normalized attn out^T
        bqk_sb = pp.tile([128, 4], F32)
        bvb_sb = pp.tile([128, C], F32)             # bv broadcast to partitions
        bob_sb = pp.tile([128, E], F32)             # bo broadcast to partitions
        ones_sb = pp.tile([1, 128], F32)

        def emit_vproj(psum_pool, st):
            """Project V for s-tile st into v_sb (with per-head ones column)."""
            ps = psum_pool.tile([128, C], F32, tag="mps", name=f"vp{st}")
            for ec in range(NEC):
                nc.tensor.matmul(
                    ps[:],
                    xt_sb[:, ec * S + st * 128: ec * S + st * 128 + 128],
                    wvt_sb[:, ec * C: (ec + 1) * C],
                    start=(ec == 0), stop=(ec == NEC - 1),
                    skip_group_check=True)
            dst = v_sb[:, st * VW: st * VW + VW].rearrange(
                "p (h x) -> p h x", h=HPC)[:, :, 0:DH]
            nc.vector.tensor_add(
                dst,
                ps[:].rearrange("p (h x) -> p h x", h=HPC),
                bvb_sb[:].rearrange("p (h x) -> p h x", h=HPC))

        with ctxpool(name="qkw", bufs=1) as qkw, \
             ctxpool(name="small", bufs=1) as sp:
            wqt_sb = qkw.tile([128, NEC * C], F32R)
            wkt_sb = qkw.tile([128, NEC * C], F32R)
            bv_row = sp.tile([1, C], F32)
            bo_row = sp.tile([1, E], F32)

            # ---- input DMAs (tiny tensors first, then interleaved chunks
            # so the PE can start after the first chunk lands) ----
            nc.sync.dma_start(out=bqk_sb[:], in_=bqk_d[:])
            nc.sync.dma_start(out=ones_sb[:], in_=ones_d[:])
            nc.sync.dma_start(out=bv_row[:], in_=bv_d[:])
            nc.sync.dma_start(out=bo_row[:], in_=bo_d[:])
            v_ones_ap = v_sb[:].rearrange("p (n x) -> p n x", x=DH + 1)[:, :, DH:DH + 1]
            nc.sync.dma_start(
                out=v_ones_ap,
                in_=onesv_d[:].bitcast(F32R).rearrange("p (n x) -> p n x", x=1))
            for ec in range(NEC):
                nc.sync.dma_start(out=xt_sb[:, ec * S:(ec + 1) * S],
                                  in_=xt_d[ec * 128:(ec + 1) * 128, :].bitcast(F32R))
                nc.sync.dma_start(out=wqt_sb[:, ec * C:(ec + 1) * C],
                                  in_=wqt_d[ec * 128:(ec + 1) * 128, :].bitcast(F32R))
                nc.sync.dma_start(out=wkt_sb[:, ec * C:(ec + 1) * C],
                                  in_=wkt_d[ec * 128:(ec + 1) * 128, :].bitcast(F32R))
            for ec in range(NEC):
                nc.sync.dma_start(out=wvt_sb[:, ec * C:(ec + 1) * C],
                                  in_=wvt_d[ec * 128:(ec + 1) * 128, :].bitcast(F32R))
            for cc in range(2):
                nc.sync.dma_start(out=wot_sb[:, cc * E:(cc + 1) * E],
                                  in_=wot_d[cc * 128:(cc + 1) * 128, :].bitcast(F32R))

            # ==== phase B: Q^T/K^T projections (e-chunk outer, 8 live
            # accumulation groups; PE paced by the DMA stream) ====
            with ctxpool(name="proj_ps", bufs=8, space="PSUM") as proj_ps:
                for dt in range(2):
                    pss = {}
                    for pj in range(2):
                        for sb_i in range(NQB):
                            pss[pj, sb_i] = proj_ps.tile(
                                [128, 512], F32, tag="pps",
                                name=f"pp_{dt}_{pj}_{sb_i}")
                    for ec in range(NEC):
                        for pj, w_sb in ((0, wqt_sb), (1, wkt_sb)):
                            for sb_i in range(NQB):
                                nc.tensor.matmul(
                                    pss[pj, sb_i][:],
                                    w_sb[:, ec * C + dt * 128:
                                         ec * C + dt * 128 + 128],
                                    xt_sb[:, ec * S + sb_i * 512:
                                          ec * S + sb_i * 512 + 512],
                                    start=(ec == 0), stop=(ec == NEC - 1),
                                    skip_group_check=True)
                    for pj, o_sb, bcol in ((0, qt_sb, 0), (1, kt_sb, 2)):
                        for sb_i in range(NQB):
                            nc.vector.tensor_scalar_add(
                                o_sb[:, dt * S + sb_i * 512:
                                     dt * S + sb_i * 512 + 512],
                                pss[pj, sb_i][:],
                                bqk_sb[:, bcol + dt: bcol + dt + 1])

            # ==== phase C: attention (q-block outer, head inner) + out-proj ====
            with ctxpool(name="score_ps", bufs=2, space="PSUM") as score_ps, \
                 ctxpool(name="attn_ps", bufs=2, space="PSUM") as attn_ps, \
                 ctxpool(name="misc_ps", bufs=2, space="PSUM") as misc_ps, \
                 ctxpool(name="pt_pool", bufs=4) as pt_pool, \
                 ctxpool(name="rec_pool", bufs=2) as rec_pool, \
                 ctxpool(name="bc_pool", bufs=2) as bc_pool, \
                 ctxpool(name="out_pool", bufs=2) as out_pool:
                # bias broadcasts via K=1 fp32 matmul against ones (inputs
                # arrived long ago; PE reaches these right after projections)
                ps_bv = misc_ps.tile([128, C], F32, tag="mps")
                nc.tensor.matmul(ps_bv[:], ones_sb[0:1, 0:128], bv_row[:],
                                 start=True, stop=True)
                nc.vector.tensor_copy(bvb_sb[:], ps_bv[:])
                for eb in range(2):
                    ps_bo = misc_ps.tile([128, 512], F32, tag="mps",
                                         name=f"bo{eb}")
                    nc.tensor.matmul(ps_bo[:], ones_sb[0:1, 0:128],
                                     bo_row[0:1, eb * 512:(eb + 1) * 512],
                                     start=True, stop=True)
                    nc.vector.tensor_copy(bob_sb[:, eb * 512:(eb + 1) * 512],
                                          ps_bo[:])

                # V for the first q-block
                for st in range(4):
                    emit_vproj(misc_ps, st)

                pending_norm = []  # deferred normalization closures

                def flush_norms():
                    while pending_norm:
                        pending_norm.pop(0)()

                for qb in range(NQB):
                    nk = 4 * (qb + 1) if causal else NST
                    q0 = qb * 512
                    for h in range(HPC):
                        t = h // 2
                        p0 = (h % 2) * 64
                        ps_o = attn_ps.tile([65, 512], F32, tag="po", name=f"po{qb}{h}")
                        for pr in range(nk // 2):
                            ps_s = score_ps.tile([128, 1024], F32, tag="sc",
                                                 name=f"sc{qb}{h}{pr}")
                            pt = pt_pool.tile([128, 1024], F32R, tag="pt",
                                              name=f"pt{qb}{h}{pr}")
                            for u in range(2):
                                kt_i = 2 * pr + u
                                nc.tensor.matmul(
                                    ps_s[:, u * 512:(u + 1) * 512],
                                    kt_sb[p0:p0 + 64,
                                          t * S + kt_i * 128: t * S + kt_i * 128 + 128],
                                    qt_sb[p0:p0 + 64, t * S + q0: t * S + q0 + 512],
                                    start=True, stop=True)
                            nc.scalar.activation(pt[:], ps_s[:], ActF.Exp,
                                                 scale=SCALE)
                            if causal and (2 * pr + 1) * 128 - q0 + 127 >= 0:
                                # single fused causal mask over both halves:
                                # keep iff q - k - (256*pr + 128*u - q0) >= 0
                                nc.gpsimd.affine_select(
                                    out=pt[:].rearrange("p (u q) -> p u q", u=2),
                                    in_=pt[:].rearrange("p (u q) -> p u q", u=2),
                                    compare_op=Alu.is_ge,
                                    fill=0.0, base=q0 - 256 * pr,
                                    pattern=[[-128, 2], [1, 512]],
                                    channel_multiplier=-1)
                            if pr == 0:
                                flush_norms()
                            for u in range(2):
                                kt_i = 2 * pr + u
                                nc.tensor.matmul(
                                    ps_o[:],
                                    v_sb[:, kt_i * VW + h * (DH + 1):
                                         kt_i * VW + h * (DH + 1) + DH + 1],
                                    pt[:, u * 512:(u + 1) * 512],
                                    start=(kt_i == 0), stop=(kt_i == nk - 1),
                                    skip_group_check=True)
                        # rowsum -> sbuf now; broadcast + 1/x + mul deferred so
                        # the next head's matmuls aren't stuck behind them
                        rs = rec_pool.tile([1, 512], F32, tag="rs", name=f"rs{qb}{h}")
                        nc.vector.tensor_copy(rs[:], ps_o[64:65, :])

                        def norm(ps_o=ps_o, rs=rs, p0=p0, t=t, q0=q0, qb=qb, h=h):
                            ps_b = misc_ps.tile([64, 512], F32, tag="mps",
                                                name=f"pb{qb}{h}")
                            nc.tensor.matmul(ps_b[:], ones_sb[0:1, 0:64], rs[:],
                                             start=True, stop=True)
                            bc = bc_pool.tile([64, 512], F32, tag="bc", name=f"bc{qb}{h}")
                            nc.vector.reciprocal_approx_fast(bc[:], ps_b[:])
                            nc.vector.tensor_mul(
                                ot_sb[p0:p0 + 64, t * S + q0: t * S + q0 + 512],
                                ps_o[0:64, :], bc[:])
                        pending_norm.append(norm)
                    flush_norms()

                    # V projection for the next q-block (PE filler during softmax)
                    if qb + 1 < NQB:
                        for st in range(4 * (qb + 1), 4 * (qb + 2)):
                            emit_vproj(misc_ps, st)

                    # out-projection for this q-block's 4 s-tiles
                    for st in range(qb * 4, qb * 4 + 4):
                        o_t = out_pool.tile([128, E], F32, tag="ob", name=f"ot{st}")
                        for eb in range(2):
                            ps_f = misc_ps.tile([128, 512], F32, tag="mps",
                                                name=f"pf{st}{eb}")
                            for cc in range(2):
                                nc.tensor.matmul(
                                    ps_f[:],
                                    ot_sb[:, cc * S + st * 128: cc * S + st * 128 + 128],
                                    wot_sb[:, cc * E + eb * 512: cc * E + eb * 512 + 512],
                                    start=(cc == 0), stop=(cc == 1))
                            nc.vector.tensor_add(
                                o_t[:, eb * 512:(eb + 1) * 512], ps_f[:],
                                bob_sb[:, eb * 512:(eb + 1) * 512])
                        nc.sync.dma_start(
                            out=out_d[st * 128:(st + 1) * 128, :], in_=o_t[:])


def _build(causal):
    nc = bacc.Bacc("TRN2", target_bir_lowering=False, debug=False,
                   num_devices=NCORES)
    with tile.TileContext(nc) as tc:
        _emit(nc, tc, causal)
    nc.compile()
    return nc


def _shard_inputs(QKV, Wq, bq, Wk, bk, Wv, bv, Wo, bo):
    QKV = np.asarray(QKV, dtype=np.float32)
    Wq, Wk, Wv, Wo = (np.asarray(w, dtype=np.float32) for w in (Wq, Wk, Wv, Wo))
    bq, bk, bv, bo = (np.asarray(b_, dtype=np.float32) for b_ in (bq, bk, bv, bo))
    ones = np.ones((1, 128), dtype=np.float32)
    onesv = np.ones((128, NST * HPC), dtype=np.float32)
    in_maps = []
    for core in range(NCORES):
        b, g = divmod(core, TPW)
        cs = slice(g * C, (g + 1) * C)
        bqs, bks = bq[cs], bk[cs]
        bqk = np.stack([bqs[:128], bqs[128:], bks[:128], bks[128:]], axis=1)
        in_maps.append({
            "xt": np.ascontiguousarray(QKV[b].T),
            "wqt": np.ascontiguousarray(Wq[cs, :].T),
            "wkt": np.ascontiguousarray(Wk[cs, :].T),
            "wvt": np.ascontiguousarray(Wv[cs, :].T),
            "wot": np.ascontiguousarray(Wo[:, cs].T),
            "bqk": np.ascontiguousarray(bqk),
            "bv": bv[cs].reshape(1, C).copy(),
            # host sums the 4 tensor-parallel partials per batch; only one
            # core per group contributes the output bias
            "bo": (bo if g == 0 else np.zeros_like(bo)).reshape(1, E).copy(),
            "ones": ones,
            "onesv": onesv,
        })
    return in_maps


def kernel(QKV, Wq, bq, Wk, bk, Wv, bv, Wo, bo, is_causal):
    causal = bool(int(np.asarray(is_causal)))
    if causal not in _cache:
        _cache[causal] = _build(causal)
    nc = _cache[causal]
    in_maps = _shard_inputs(QKV, Wq, bq, Wk, bk, Wv, bv, Wo, bo)
    res = run_bass_kernel_spmd(nc, in_maps, core_ids=list(range(NCORES)))
    out = np.empty((B, S, E), dtype=np.float32)
    for b in range(B):
        acc = res.results[TPW * b]["out"].astype(np.float32)
        for g in range(1, TPW):
            acc = acc + res.results[TPW * b + g]["out"]
        out[b] = acc
    return out


# revision 12
# speedup vs baseline: 1.2544x; 1.0909x over previous
"""Multi-head attention (B=2, S=2048, E=1024, H=16, causal) on 8 Trainium2 cores.

Sharding: data-parallel over batch (2) x tensor-parallel over heads (4 groups
of 4 heads). Core i handles batch i//4, heads 4*(i%4) .. 4*(i%4)+3.
Each core computes Q/K/V projections for its 256 channels, causal
flash-attention for its 4 heads, and a partial output projection
(contribution of its channels to all 1024 output features). Partials are
summed across the 4 cores of each batch group (host-side).

All big matmuls run as float32r (full-rate on TRN2 TensorE for N>=256).
V projection is deferred per-q-block to give the PE filler work while the
ACT engine paces the softmax inner loop (keeps HAM from throttling).
"""
import numpy as np

import concourse.bass as bass
import concourse.tile as tile
from concourse import bacc, mybir
from concourse.bass_utils import run_bass_kernel_spmd

F32 = mybir.dt.float32
F32R = mybir.dt.float32r
ActF = mybir.ActivationFunctionType
Alu = mybir.AluOpType

B, S, E = 2, 2048, 1024
H, DH = 16, 64
NCORES, TPW = 8, 4          # 8 cores, 4-way tensor parallel per batch
HPC = H // TPW              # heads per core = 4
C = HPC * DH                # channels per core = 256
SCALE = 1.0 / 8.0           # 1/sqrt(DH)
VW = HPC * (DH + 1)         # V storage width per s-tile (ones col per head)
NST = S // 128              # 16 s-tiles of 128 rows
NQB = S // 512              # 4 q-blocks of 512
NEC = E // 128              # 8 e-chunks (contraction for projections)

_cache = {}


def _emit(nc, tc, causal):
    # ---- DRAM parameters ----
    xt_d = nc.dram_tensor("xt", [E, S], F32, kind="ExternalInput").ap()
    wqt_d = nc.dram_tensor("wqt", [E, C], F32, kind="ExternalInput").ap()
    wkt_d = nc.dram_tensor("wkt", [E, C], F32, kind="ExternalInput").ap()
    wvt_d = nc.dram_tensor("wvt", [E, C], F32, kind="ExternalInput").ap()
    wot_d = nc.dram_tensor("wot", [C, E], F32, kind="ExternalInput").ap()
    bqk_d = nc.dram_tensor("bqk", [128, 4], F32, kind="ExternalInput").ap()
    bv_d = nc.dram_tensor("bv", [1, C], F32, kind="ExternalInput").ap()
    bo_d = nc.dram_tensor("bo", [1, E], F32, kind="ExternalInput").ap()
    ones_d = nc.dram_tensor("ones", [1, 128], F32, kind="ExternalInput").ap()
    onesv_d = nc.dram_tensor("onesv", [128, NST * HPC], F32, kind="ExternalInput").ap()
    out_d = nc.dram_tensor("out", [S, E], F32, kind="ExternalOutput").ap()

    ctxpool = tc.tile_pool

    with ctxpool(name="persist", bufs=1) as pp:
        # ---- persistent SBUF tensors ----
        xt_sb = pp.tile([128, NEC * S], F32R)       # X^T, e-chunk ec at cols [ec*S)
        wvt_sb = pp.tile([128, NEC * C], F32R)
        wot_sb = pp.tile([128, 2 * E], F32R)        # c-chunk cc at cols [cc*E)
        qt_sb = pp.tile([128, 2 * S], F32R)         # Q^T, d-tile t at cols [t*S)
        kt_sb = pp.tile([128, 2 * S], F32R)
        v_sb = pp.tile([128, NST * VW], F32R)       # V (+ones col per head)
        ot_sb = pp.tile([128, 2 * S], F32R)         # normalized attn out^T
        bqk_sb = pp.tile([128, 4], F32)
        bvb_sb = pp.tile([128, C], F32)             # bv broadcast to partitions
        bob_sb = pp.tile([128, E], F32)             # bo broadcast to partitions
        ones_sb = pp.tile([1, 128], F32)

        def emit_vproj(psum_pool, st):
            """Project V for s-tile st into v_sb (with per-head ones column)."""
            ps = psum_pool.tile([128, C], F32, tag="mps", name=f"vp{st}")
            for ec in range(NEC):
                nc.tensor.matmul(
                    ps[:],
                    xt_sb[:, ec * S + st * 128: ec * S + st * 128 + 128],
                    wvt_sb[:, ec * C: (ec + 1) * C],
                    start=(ec == 0), stop=(ec == NEC - 1),
                    skip_group_check=True)
            dst = v_sb[:, st * VW: st * VW + VW].rearrange(
                "p (h x) -> p h x", h=HPC)[:, :, 0:DH]
            nc.vector.tensor_add(
                dst,
                ps[:].rearrange("p (h x) -> p h x", h=HPC),
                bvb_sb[:].rearrange("p (h x) -> p h x", h=HPC))

        with ctxpool(name="qkw", bufs=1) as qkw, \
             ctxpool(name="small", bufs=1) as sp:
            wqt_sb = qkw.tile([128, NEC * C], F32R)
            wkt_sb = qkw.tile([128, NEC * C], F32R)
            bv_row = sp.tile([1, C], F32)
            bo_row = sp.tile([1, E], F32)

            # ---- input DMAs (tiny tensors first, then interleaved chunks
            # so the PE can start after the first chunk lands) ----
            nc.sync.dma_start(out=bqk_sb[:], in_=bqk_d[:])
            nc.sync.dma_start(out=ones_sb[:], in_=ones_d[:])
            nc.sync.dma_start(out=bv_row[:], in_=bv_d[:])
            nc.sync.dma_start(out=bo_row[:], in_=bo_d[:])
            v_ones_ap = v_sb[:].rearrange("p (n x) -> p n x", x=DH + 1)[:, :, DH:DH + 1]
            nc.sync.dma_start(
                out=v_ones_ap,
                in_=onesv_d[:].bitcast(F32R).rearrange("p (n x) -> p n x", x=1))
            for ec in range(NEC):
                for sq in range(4):
                    nc.sync.dma_start(
                        out=xt_sb[:, ec * S + sq * 512: ec * S + (sq + 1) * 512],
                        in_=xt_d[ec * 128:(ec + 1) * 128,
                                 sq * 512:(sq + 1) * 512].bitcast(F32R))
                nc.sync.dma_start(out=wqt_sb[:, ec * C:(ec + 1) * C],
                                  in_=wqt_d[ec * 128:(ec + 1) * 128, :].bitcast(F32R))
                nc.sync.dma_start(out=wkt_sb[:, ec * C:(ec + 1) * C],
                                  in_=wkt_d[ec * 128:(ec + 1) * 128, :].bitcast(F32R))
            for ec in range(NEC):
                nc.sync.dma_start(out=wvt_sb[:, ec * C:(ec + 1) * C],
                                  in_=wvt_d[ec * 128:(ec + 1) * 128, :].bitcast(F32R))
            for cc in range(2):
                nc.sync.dma_start(out=wot_sb[:, cc * E:(cc + 1) * E],
                                  in_=wot_d[cc * 128:(cc + 1) * 128, :].bitcast(F32R))

            # ==== phase B: Q^T/K^T projections (e-chunk outer, 8 live
            # accumulation groups; PE paced by the DMA stream) ====
            with ctxpool(name="proj_ps", bufs=8, space="PSUM") as proj_ps:
                for dt in range(2):
                    pss = {}
                    for pj in range(2):
                        for sb_i in range(NQB):
                            pss[pj, sb_i] = proj_ps.tile(
                                [128, 512], F32, tag="pps",
                                name=f"pp_{dt}_{pj}_{sb_i}")
                    for ec in range(NEC):
                        for pj, w_sb in ((0, wqt_sb), (1, wkt_sb)):
                            for sb_i in range(NQB):
                                nc.tensor.matmul(
                                    pss[pj, sb_i][:],
                                    w_sb[:, ec * C + dt * 128:
                                         ec * C + dt * 128 + 128],
                                    xt_sb[:, ec * S + sb_i * 512:
                                          ec * S + sb_i * 512 + 512],
                                    start=(ec == 0), stop=(ec == NEC - 1),
                                    skip_group_check=True)
                    for pj, o_sb, bcol in ((0, qt_sb, 0), (1, kt_sb, 2)):
                        for sb_i in range(NQB):
                            nc.vector.tensor_scalar_add(
                                o_sb[:, dt * S + sb_i * 512:
                                     dt * S + sb_i * 512 + 512],
                                pss[pj, sb_i][:],
                                bqk_sb[:, bcol + dt: bcol + dt + 1])

            # ==== phase C: attention (q-block outer, head inner) + out-proj ====
            with ctxpool(name="score_ps", bufs=2, space="PSUM") as score_ps, \
                 ctxpool(name="attn_ps", bufs=3, space="PSUM") as attn_ps, \
                 ctxpool(name="misc_ps", bufs=1, space="PSUM") as misc_ps, \
                 ctxpool(name="pt_pool", bufs=5) as pt_pool, \
                 ctxpool(name="rec_pool", bufs=2) as rec_pool, \
                 ctxpool(name="bc_pool", bufs=2) as bc_pool, \
                 ctxpool(name="out_pool", bufs=2) as out_pool:
                # bias broadcasts via K=1 fp32 matmul against ones (inputs
                # arrived long ago; PE reaches these right after projections)
                ps_bv = misc_ps.tile([128, C], F32, tag="mps")
                nc.tensor.matmul(ps_bv[:], ones_sb[0:1, 0:128], bv_row[:],
                                 start=True, stop=True)
                nc.vector.tensor_copy(bvb_sb[:], ps_bv[:])
                for eb in range(2):
                    ps_bo = misc_ps.tile([128, 512], F32, tag="mps",
                                         name=f"bo{eb}")
                    nc.tensor.matmul(ps_bo[:], ones_sb[0:1, 0:128],
                                     bo_row[0:1, eb * 512:(eb + 1) * 512],
                                     start=True, stop=True)
                    nc.vector.tensor_copy(bob_sb[:, eb * 512:(eb + 1) * 512],
                                          ps_bo[:])

                # V for the first q-block
                for st in range(4):
                    emit_vproj(misc_ps, st)

                pending_norm = []  # deferred normalization closures

                def flush_norms():
                    while pending_norm:
                        pending_norm.pop(0)()

                for qb in range(NQB):
                    nk = 4 * (qb + 1) if causal else NST
                    q0 = qb * 512
                    for h in range(HPC):
                        t = h // 2
                        p0 = (h % 2) * 64
                        ps_o = attn_ps.tile([65, 512], F32, tag="po", name=f"po{qb}{h}")
                        for pr in range(nk // 2):
                            ps_s = score_ps.tile([128, 1024], F32, tag="sc",
                                                 name=f"sc{qb}{h}{pr}")
                            pt = pt_pool.tile([128, 1024], F32R, tag="pt",
                                              name=f"pt{qb}{h}{pr}")
                            for u in range(2):
                                kt_i = 2 * pr + u
                                nc.tensor.matmul(
                                    ps_s[:, u * 512:(u + 1) * 512],
                                    kt_sb[p0:p0 + 64,
                                          t * S + kt_i * 128: t * S + kt_i * 128 + 128],
                                    qt_sb[p0:p0 + 64, t * S + q0: t * S + q0 + 512],
                                    start=True, stop=True)
                            nc.scalar.activation(pt[:], ps_s[:], ActF.Exp,
                                                 scale=SCALE)
                            if causal and (2 * pr + 1) * 128 - q0 + 127 >= 0:
                                # single fused causal mask over both halves:
                                # keep iff q - k - (256*pr + 128*u - q0) >= 0
                                nc.gpsimd.affine_select(
                                    out=pt[:].rearrange("p (u q) -> p u q", u=2),
                                    in_=pt[:].rearrange("p (u q) -> p u q", u=2),
                                    compare_op=Alu.is_ge,
                                    fill=0.0, base=q0 - 256 * pr,
                                    pattern=[[-128, 2], [1, 512]],
                                    channel_multiplier=-1)
                            if pr == 0:
                                flush_norms()
                            for u in range(2):
                                kt_i = 2 * pr + u
                                nc.tensor.matmul(
                                    ps_o[:],
                                    v_sb[:, kt_i * VW + h * (DH + 1):
                                         kt_i * VW + h * (DH + 1) + DH + 1],
                                    pt[:, u * 512:(u + 1) * 512],
                                    start=(kt_i == 0), stop=(kt_i == nk - 1),
                                    skip_group_check=True)
                        # rowsum -> sbuf now; broadcast + 1/x + mul deferred so
                        # the next head's matmuls aren't stuck behind them
                        rs = rec_pool.tile([1, 512], F32, tag="rs", name=f"rs{qb}{h}")
                        nc.vector.tensor_copy(rs[:], ps_o[64:65, :])

                        def norm(ps_o=ps_o, rs=rs, p0=p0, t=t, q0=q0, qb=qb, h=h):
                            ps_b = attn_ps.tile([64, 512], F32, tag="po",
                                                name=f"pb{qb}{h}")
                            nc.tensor.matmul(ps_b[:], ones_sb[0:1, 0:64], rs[:],
                                             start=True, stop=True)
                            bc = bc_pool.tile([64, 512], F32, tag="bc", name=f"bc{qb}{h}")
                            nc.vector.reciprocal_approx_fast(bc[:], ps_b[:])
                            nc.vector.tensor_mul(
                                ot_sb[p0:p0 + 64, t * S + q0: t * S + q0 + 512],
                                ps_o[0:64, :], bc[:])
                        pending_norm.append(norm)
                    flush_norms()

                    # V projection for the next q-block (PE filler during softmax)
                    if qb + 1 < NQB:
                        for st in range(4 * (qb + 1), 4 * (qb + 2)):
                            emit_vproj(misc_ps, st)

                    # out-projection for this q-block's 4 s-tiles
                    for st in range(qb * 4, qb * 4 + 4):
                        o_t = out_pool.tile([128, E], F32, tag="ob", name=f"ot{st}")
                        for eb in range(2):
                            ps_f = misc_ps.tile([128, 512], F32, tag="mps",
                                                name=f"pf{st}{eb}")
                            for cc in range(2):
                                nc.tensor.matmul(
                                    ps_f[:],
                                    ot_sb[:, cc * S + st * 128: cc * S + st * 128 + 128],
                                    wot_sb[:, cc * E + eb * 512: cc * E + eb * 512 + 512],
                                    start=(cc == 0), stop=(cc == 1))
                            nc.vector.tensor_add(
                                o_t[:, eb * 512:(eb + 1) * 512], ps_f[:],
                                bob_sb[:, eb * 512:(eb + 1) * 512])
                        nc.sync.dma_start(
                            out=out_d[st * 128:(st + 1) * 128, :], in_=o_t[:])


def _build(causal):
    nc = bacc.Bacc("TRN2", target_bir_lowering=False, debug=False,
                   num_devices=NCORES)
    with tile.TileContext(nc) as tc:
        _emit(nc, tc, causal)
    nc.compile()
    return nc


def _shard_inputs(QKV, Wq, bq, Wk, bk, Wv, bv, Wo, bo):
    QKV = np.asarray(QKV, dtype=np.float32)
    Wq, Wk, Wv, Wo = (np.asarray(w, dtype=np.float32) for w in (Wq, Wk, Wv, Wo))
    bq, bk, bv, bo = (np.asarray(b_, dtype=np.float32) for b_ in (bq, bk, bv, bo))
    ones = np.ones((1, 128), dtype=np.float32)
    onesv = np.ones((128, NST * HPC), dtype=np.float32)
    in_maps = []
    for core in range(NCORES):
        b, g = divmod(core, TPW)
        cs = slice(g * C, (g + 1) * C)
        bqs, bks = bq[cs], bk[cs]
        bqk = np.stack([bqs[:128], bqs[128:], bks[:128], bks[128:]], axis=1)
        in_maps.append({
            "xt": np.ascontiguousarray(QKV[b].T),
            "wqt": np.ascontiguousarray(Wq[cs, :].T),
            "wkt": np.ascontiguousarray(Wk[cs, :].T),
            "wvt": np.ascontiguousarray(Wv[cs, :].T),
            "wot": np.ascontiguousarray(Wo[:, cs].T),
            "bqk": np.ascontiguousarray(bqk),
            "bv": bv[cs].reshape(1, C).copy(),
            # host sums the 4 tensor-parallel partials per batch; only one
            # core per group contributes the output bias
            "bo": (bo if g == 0 else np.zeros_like(bo)).reshape(1, E).copy(),
            "ones": ones,
            "onesv": onesv,
        })
    return in_maps


def kernel(QKV, Wq, bq, Wk, bk, Wv, bv, Wo, bo, is_causal):
    causal = bool(int(np.asarray(is_causal)))
    if causal not in _cache:
        _cache[causal] = _build(causal)
    nc = _cache[causal]
    in_maps = _shard_inputs(QKV, Wq, bq, Wk, bk, Wv, bv, Wo, bo)
    res = run_bass_kernel_spmd(nc, in_maps, core_ids=list(range(NCORES)))
    out = np.empty((B, S, E), dtype=np.float32)
    for b in range(B):
        acc = res.results[TPW * b]["out"].astype(np.float32)
        for g in range(1, TPW):
            acc = acc + res.results[TPW * b + g]["out"]
        out[b] = acc
    return out
